# revision 34
# baseline (speedup 1.0000x reference)
"""Trainium2 Bass kernel for the attention-gate block (bf16 pipeline).

Math (per sample n, after folding BN into the convs):
  X     = x[n, :, ::2, ::2].reshape(C, 4)                 # C=512, L=4
  act_k = relu(Wk' @ X + bk')            k=0,1,2          # D=64 each
  S     = act0^T act1  (4x4);  P = softmax_rows(S)
  Z     = P @ act2^T  (4x64)
  Y     = W4' @ Z^T + b4'                                  # (512, 4)
  out[n,c,h,w] = x[n,c,h,w] + Y[c,h]                       # broadcast over w

Design (per core, 256 samples, blocks of 128):
  - everything on the wire and in the matmuls is bf16 (fp32 PSUM accum):
    4x fewer PE cycles than fp32 and half the HBM traffic.  Measured
    numerical impact on the final output is fro ~3e-3 (gate is 2e-2).
  - x is repacked HOST-side to [core][p, n, (j w h)] with c = 4p + j, so
    each block DMA is 128 partitions x 16KB contiguous -- line-rate HBM.
    The in-run order (j, w, h) makes the residual add's inner dimension
    (h) match y's layout, so the broadcast-over-w add runs stride-1.
  - all constants ship in two packed blobs (2 DMAs) on the scalar (ACT)
    queue; x block loads are issued first on the sync (SP) queue (block
    0 split in halves across both queues) so compute starts early.
  - per-subchunk tiles and ops (32 samples each) keep a fine-grained
    software pipeline across PE/ACT/DVE.
  - softmax denominators are computed TRANSPOSED (lhsT=p0, rhs=ones ->
    [128,1] per sub, batched in one [128,4] bank) so a single DVE
    reciprocal runs across 128 lanes; r broadcasts back to rows via an
    identity-rhs matmul with a stride-0 lhsT.
  - GEMM2 is one 512-col matmul per channel group j; its bias rides the
    ACT PSUM->SBUF copy; the residual is a pure bf16 SBUF tensor_tensor
    add at DVE 2x mode, done per (j, n-half) so each half's store (split
    across both HWDGE queues) starts early.
"""

import os
import sys

for _p in ("/opt/trn_rl_repo",):
    if _p not in sys.path:
        sys.path.insert(0, _p)

import numpy as np
from ml_dtypes import bfloat16

import concourse.mybir as mybir
from concourse import bacc, tile

EPS = 1e-5
N_TOTAL, C, D, HH, WW = 2048, 512, 64, 4, 4
NCORES = 8
NSH = N_TOTAL // NCORES  # 256 samples per core
BLK = int(os.environ.get("KBLK", "128"))  # samples per block
SUB = 32                 # samples per attention subchunk (4*SUB = 128 cols)
SHIFT = -34.0            # constant exp shift; cancels in the normalization
F32 = mybir.dt.float32
BF16 = mybir.dt.bfloat16

# bf16 const-blob column layout
_WQ, _WK, _W2A = 0, 256, 512
_MSK, _EYE, _W4T, _B2A, _CBW = 768, 896, 1024, 1536, 1600

_PROG_CACHE = {}


def build_program(nsh=NSH, blk=BLK, reps=1):
    key = (nsh, blk, reps)
    if key in _PROG_CACHE:
        return _PROG_CACHE[key]

    nc = bacc.Bacc("TRN2", target_bir_lowering=False, debug=False)
    AF = mybir.ActivationFunctionType

    x_in = nc.dram_tensor("x", (128, nsh * 64), BF16, kind="ExternalInput")
    cb = nc.dram_tensor("cb", (128, _CBW), BF16, kind="ExternalInput")
    cf = nc.dram_tensor("cf", (128, 8), F32, kind="ExternalInput")
    out = nc.dram_tensor("out", (128, nsh * 64), BF16, kind="ExternalOutput")

    nblk = nsh // blk
    nsub = blk // SUB
    NF = 4 * blk  # free width of a full block of (n, l) columns
    NH = blk // 2  # samples per store half

    with tile.TileContext(nc) as tc:
        with (
            tc.tile_pool(name="const", bufs=1) as cpool,
            tc.tile_pool(name="xp", bufs=(3 if blk >= 128 else 4)) as xpool,
            tc.tile_pool(name="work", bufs=4) as wpool,
            tc.tile_pool(name="att", bufs=6) as apool,
            tc.tile_pool(name="ps", bufs=5, space="PSUM") as pspool,
            tc.tile_pool(name="psd", bufs=1, space="PSUM") as pdpool,
            tc.tile_pool(name="psy", bufs=2, space="PSUM") as pypool,
        ):
            # constants first (small, gate all matmuls): scalar queue
            cb_sb = cpool.tile([128, _CBW], BF16)
            nc.scalar.dma_start(cb_sb[:], cb[:])
            cf_sb = cpool.tile([128, 8], F32)
            nc.scalar.dma_start(cf_sb[:], cf[:])

            # x block loads: block 0 split across both queues; later
            # blocks go on the scalar queue so they never get ahead of
            # block 0's halves in the HBM bandwidth race
            xv = x_in[:].rearrange("p (b n f) -> b p n f", n=blk, f=64)
            ov = out[:].rearrange("p (b n f) -> b p n f", n=blk, f=64)
            blist = [b for _ in range(reps) for b in range(nblk)]
            xts = {}
            for i, b in enumerate(blist[: max(2, min(3, len(blist)))]):
                x_t = xpool.tile([128, blk, 64], BF16, tag="x")
                if i == 0:
                    nc.sync.dma_start(x_t[:, 0:blk // 2], xv[b][:, 0:blk // 2])
                    nc.scalar.dma_start(x_t[:, blk // 2:], xv[b][:, blk // 2:])
                else:
                    nc.scalar.dma_start(x_t[:], xv[b])
                xts[i] = x_t

            ones_sb = cpool.tile([1, 128], BF16)
            nc.vector.memset(ones_sb[:], 1.0)
            ones_col = cpool.tile([128, 1], BF16)
            nc.vector.memset(ones_col[:], 1.0)
            shift_sb = cpool.tile([128, 1], F32)
            nc.vector.memset(shift_sb[:], SHIFT)

            # PE warm-up: the HAM clock gate keeps the PE at 1.2 GHz until
            # it has seen ~3.4us of sustained matmul activity.  While the
            # x/weight DMAs are in flight the PE is otherwise idle, so burn
            # that window on dummy matmuls over a memset tile; the real
            # GEMMs then start at 2.4 GHz.
            warm_w = cpool.tile([128, 128], BF16)
            nc.vector.memset(warm_w[:], 0.0)
            warm_in = cpool.tile([128, 512], BF16)
            nc.vector.memset(warm_in[:], 0.0)
            ps_w = pypool.tile([128, 512], F32, tag="psy")
            for k in range(8):
                # full K=128 contraction: the activity monitor watches the
                # array, so thin matmuls never trip the clock gate
                nc.tensor.matmul(
                    ps_w[:], lhsT=warm_w[:], rhs=warm_in[:],
                    start=True, stop=True,
                )

            wqv = cb_sb[:, _WQ:_WQ + 256].rearrange("p (j d) -> p j d", j=4)
            wkv = cb_sb[:, _WK:_WK + 256].rearrange("p (j d) -> p j d", j=4)
            w2v = cb_sb[:, _W2A:_W2A + 256].rearrange("p (j d) -> p j d", j=4)
            mskv = cb_sb[:, _MSK:_MSK + 128]
            eyev = cb_sb[:, _EYE:_EYE + 128]
            w4v = cb_sb[0:D, _W4T:_W4T + 512].rearrange(
                "p (j c) -> p j c", j=4)
            b2av = cb_sb[0:1, _B2A:_B2A + D]
            bq_ap = cf_sb[0:D, 0:1]
            bk_ap = cf_sb[0:D, 1:2]

            # ---- per-block stages, software-pipelined across blocks ----

            def st_front_a(i, b):
                """Load view, gather, GEMM1 q/k + relu."""
                if i in xts:
                    x_t = xts[i]
                else:
                    x_t = xpool.tile([128, blk, 64], BF16, tag="x")
                    nc.sync.dma_start(x_t[:], xv[b])
                # in-run order is (j, w, h)
                xtv = x_t[:].rearrange("p n (j w h) -> p n j w h", j=4, w=4)
                # gather the ::2,::2 columns -> [128, j, n, l], l=(h',w')
                xr = wpool.tile([128, 4, blk, 4], BF16, tag="xr")
                xrv = xr[:].rearrange("p j n (a c) -> p j n a c", a=2)
                for hh in range(2):
                    nsl = slice(hh * NH, (hh + 1) * NH)
                    nc.vector.tensor_copy(
                        xrv[:, :, nsl],
                        xtv[:, nsl, :, 0:4:2, 0:4:2].transpose(
                            [0, 2, 1, 4, 3]),
                    )
                xrf = xr[:].rearrange("p j n l -> p j (n l)")
                ps_q = pspool.tile([D, NF], F32, tag="ps")
                ps_k = pspool.tile([D, NF], F32, tag="ps")
                for j in range(4):
                    nc.tensor.matmul(
                        ps_q[:], lhsT=wqv[:, j], rhs=xrf[:, j],
                        start=(j == 0), stop=(j == 3),
                    )
                for j in range(4):
                    nc.tensor.matmul(
                        ps_k[:], lhsT=wkv[:, j], rhs=xrf[:, j],
                        start=(j == 0), stop=(j == 3),
                    )
                a_q = wpool.tile([D, NF], BF16, tag="aq")
                nc.scalar.activation(a_q[:], ps_q[:], AF.Relu, bias=bq_ap)
                a_k = wpool.tile([D, NF], BF16, tag="ak")
                nc.scalar.activation(a_k[:], ps_k[:], AF.Relu, bias=bk_ap)
                return dict(b=b, x_t=x_t, xtv=xtv, xrf=xrf, a_q=a_q, a_k=a_k)

            def st_front_b(st):
                """Phase 1: vt + gram matmuls for all subchunks."""
                xrf, a_q, a_k = st["xrf"], st["a_q"], st["a_k"]
                ph_vt, ph_g = [], []
                for s in range(nsub):
                    cl = slice(s * 128, s * 128 + 128)
                    ps_vt = pspool.tile([128, D], F32, tag="ps")
                    for j in range(4):
                        nc.tensor.matmul(
                            ps_vt[:], lhsT=xrf[:, j, cl], rhs=w2v[:, j],
                            start=(j == 0), stop=False,
                        )
                    nc.tensor.matmul(
                        ps_vt[:], lhsT=ones_sb[:], rhs=b2av,
                        start=False, stop=True,
                    )
                    ps_g = pspool.tile([128, 128], F32, tag="ps")
                    nc.tensor.matmul(
                        ps_g[:], lhsT=a_k[:, cl], rhs=a_q[:, cl],
                        start=True, stop=True,
                    )
                    ph_vt.append(ps_vt)
                    ph_g.append(ps_g)
                st["ph_vt"], st["ph_g"] = ph_vt, ph_g

            def st_attn(st):
                """Phase 2 + denominators + phase 3 -> normalized z."""
                ph_vt, ph_g = st["ph_vt"], st["ph_g"]
                ph_a2t, ph_p0 = [], []
                for s in range(nsub):
                    a2t = apool.tile([128, D], BF16, tag="a2t")
                    nc.scalar.activation(a2t[:], ph_vt[s][:], AF.Relu)
                    e_t = apool.tile([128, 128], BF16, tag="e")
                    nc.scalar.activation(e_t[:], ph_g[s][:], AF.Exp,
                                         bias=shift_sb[:])
                    p0 = apool.tile([128, 128], BF16, tag="p0")
                    nc.vector.tensor_mul(p0[:], e_t[:], mskv)
                    ph_a2t.append(a2t)
                    ph_p0.append(p0)
                ps_dt = pdpool.tile([128, nsub], F32, tag="psd")
                for s in range(nsub):
                    nc.tensor.matmul(
                        ps_dt[:, s:s + 1], lhsT=ph_p0[s][:], rhs=ones_col[:],
                        start=(s == 0), stop=(s == nsub - 1),
                    )
                rt_sb = apool.tile([128, nsub], BF16, tag="rt")
                with nc.allow_low_precision("bf16 softmax normalization"):
                    nc.vector.reciprocal(rt_sb[:], ps_dt[:])
                z_blk = apool.tile([D, NF], BF16, tag="z")
                for s in range(nsub):
                    a2t, p0 = ph_a2t[s], ph_p0[s]
                    ps_z = pspool.tile([D, 128], F32, tag="ps")
                    nc.tensor.matmul(
                        ps_z[:], lhsT=a2t[:], rhs=p0[:], start=True, stop=True,
                    )
                    ps_r = pspool.tile([D, 128], F32, tag="ps")
                    nc.tensor.matmul(
                        ps_r[:],
                        lhsT=rt_sb[:, s:s + 1].broadcast_to((128, D)),
                        rhs=eyev,
                        start=True, stop=True,
                    )
                    r64_sb = apool.tile([D, 128], BF16, tag="r64")
                    nc.scalar.activation(r64_sb[:], ps_r[:], AF.Copy)
                    nc.vector.tensor_mul(
                        z_blk[:, s * 128:(s + 1) * 128], ps_z[:], r64_sb[:])
                st["z_blk"] = z_blk

            def st_out(st):
                """GEMM2 + bias copy + residual + stores."""
                b, x_t, xtv, z_blk = st["b"], st["x_t"], st["xtv"], st["z_blk"]
                y4 = apool.tile([128, 4, NF], BF16, tag="y")
                for j in range(4):
                    ps_y = pypool.tile([128, NF], F32, tag="psy")
                    nc.tensor.matmul(
                        ps_y[:], lhsT=w4v[:, j], rhs=z_blk[:],
                        start=True, stop=True,
                    )
                    nc.scalar.activation(y4[:, j], ps_y[:], AF.Identity,
                                         bias=cf_sb[:, 4 + j:5 + j])
                y4v = y4[:].rearrange("p j (n l) -> p n j l", l=4)
                for hh in range(2):
                    nsl = slice(hh * NH, (hh + 1) * NH)
                    for j in range(4):
                        yv = (
                            y4v[:, nsl, j]
                            .unsqueeze(2)
                            .broadcast_to((128, NH, 4, 4))
                        )
                        nc.vector.tensor_add(xtv[:, nsl, j], yv,
                                             xtv[:, nsl, j])
                    eng = nc.sync if hh == 0 else nc.scalar
                    eng.dma_start(ov[b][:, nsl], x_t[:, nsl])

            # pipelined emission: block i+1's front overlaps block i's
            # attention/output stages
            sts = {}
            for i, b in enumerate(blist):
                if i == 0:
                    sts[0] = st_front_a(0, b)
                    st_front_b(sts[0])
                if i + 1 < len(blist):
                    sts[i + 1] = st_front_a(i + 1, blist[i + 1])
                st_attn(sts[i])
                if i + 1 < len(blist):
                    st_front_b(sts[i + 1])
                st_out(sts[i])
                del sts[i]

    nc.compile()
    _PROG_CACHE[key] = nc
    return nc


def _bf16(a):
    return np.ascontiguousarray(np.asarray(a, np.float32)).astype(bfloat16)


def prep_params(W123, b123, g123, be123, m123, v123, W4, b4, g4, be4, m4, v4):
    """Fold BN into the convs; pack into the two const blobs."""
    f64 = np.float64
    s123 = (g123 / np.sqrt(v123 + EPS)).astype(f64)            # (3, D)
    Wf = (W123 * s123[:, :, None]).astype(f64)                 # (3, D, C)
    bf = ((b123 - m123) * s123 + be123).astype(np.float32)     # (3, D)
    s4 = (g4 / np.sqrt(v4 + EPS)).astype(f64)                  # (C,)
    W4f = (W4 * s4[:, None]).astype(f64)                       # (C, D)
    b4f = ((b4 - m4) * s4 + be4).astype(np.float32)            # (C,)

    # perm[j*128 + p] = 4p + j : row j*128+p of a device weight tensor
    # holds original channel 4p+j (matching the x packing).
    p_idx, j_idx = np.meshgrid(np.arange(128), np.arange(4), indexing="ij")
    perm = (4 * p_idx + j_idx).T.reshape(-1)                   # (512,)

    def to_pjd(w):  # (D, C) weights -> [128, (j, D)] with c = 4p+j
        return w.T[perm].reshape(4, 128, D).transpose(1, 0, 2).reshape(128, -1)

    msk = np.kron(np.eye(SUB, dtype=np.float32), np.ones((4, 4), np.float32))
    cbv = np.zeros((128, _CBW), np.float32)
    cbv[:, _WQ:_WQ + 256] = to_pjd(Wf[0])
    cbv[:, _WK:_WK + 256] = to_pjd(Wf[1])
    cbv[:, _W2A:_W2A + 256] = to_pjd(Wf[2])
    cbv[:, _MSK:_MSK + 128] = msk
    cbv[:, _EYE:_EYE + 128] = np.eye(128, dtype=np.float32)
    cbv[0:D, _W4T:_W4T + 512] = W4f.T[:, perm].reshape(D, 512)
    cbv[0, _B2A:_B2A + D] = bf[2]
    cfv = np.zeros((128, 8), np.float32)
    cfv[0:D, 0] = bf[0]
    cfv[0:D, 1] = bf[1]
    cfv[:, 4:8] = b4f[perm].reshape(4, 128).T
    return dict(cb=_bf16(cbv), cf=cfv)


def pack_x(x):
    """(N, C, 4, 4) fp32 -> [NCORES][128, NSH*64] bf16.

    c = 4p+j on partitions; per-(p, n) run holds (j, w, h)."""
    xb = np.asarray(x, np.float32).reshape(NCORES, NSH, 128, 4, 4, 4)
    xb = xb.astype(bfloat16).transpose(0, 2, 1, 3, 5, 4)  # core p n j w h
    return np.ascontiguousarray(xb).reshape(NCORES, 128, NSH * 64)


def unpack_out(outs):
    """[NCORES][128, NSH*64] bf16 -> (N, C, 4, 4) fp32."""
    o = np.stack([np.asarray(c) for c in outs]).reshape(
        NCORES, 128, NSH, 4, 4, 4)               # core p n j w h
    o = o.transpose(0, 2, 1, 3, 5, 4)            # core n p j h w
    return np.ascontiguousarray(o).reshape(N_TOTAL, C, HH, WW).astype(
        np.float32)


def _run(inputs, trace=False, **spmd_kwargs):
    from concourse.bass_utils import run_bass_kernel_spmd

    xp = pack_x(inputs["x"])
    params = prep_params(**{k: np.asarray(v, np.float64)
                            for k, v in inputs.items() if k != "x"})
    nc = build_program()
    in_maps = [{"x": xp[i], **params} for i in range(NCORES)]
    res = run_bass_kernel_spmd(
        nc, in_maps, list(range(NCORES)), trace=trace, **spmd_kwargs
    )
    outs = unpack_out([res.results[i]["out"] for i in range(NCORES)])
    return outs, res


def kernel(**inputs):
    outs, _ = _run(inputs)
    return outs


# revision 35
# speedup vs baseline: 1.0657x; 1.0657x over previous
"""Trainium2 Bass kernel for the attention-gate block (bf16 pipeline).

Math (per sample n, after folding BN into the convs):
  X     = x[n, :, ::2, ::2].reshape(C, 4)                 # C=512, L=4
  act_k = relu(Wk' @ X + bk')            k=0,1,2          # D=64 each
  S     = act0^T act1  (4x4);  P = softmax_rows(S)
  Z     = P @ act2^T  (4x64)
  Y     = W4' @ Z^T + b4'                                  # (512, 4)
  out[n,c,h,w] = x[n,c,h,w] + Y[c,h]                       # broadcast over w

Design (per core, 256 samples, blocks of 128):
  - everything on the wire and in the matmuls is bf16 (fp32 PSUM accum):
    4x fewer PE cycles than fp32 and half the HBM traffic.  Measured
    numerical impact on the final output is fro ~3e-3 (gate is 2e-2).
  - x is repacked HOST-side to [core][p, n, (j w h)] with c = 4p + j, so
    each block DMA is 128 partitions x 16KB contiguous -- line-rate HBM.
    The in-run order (j, w, h) makes the residual add's inner dimension
    (h) match y's layout, so the broadcast-over-w add runs stride-1.
  - all constants ship in two packed blobs (2 DMAs) on the scalar (ACT)
    queue; x block loads are issued first on the sync (SP) queue (block
    0 split in halves across both queues) so compute starts early.
  - per-subchunk tiles and ops (32 samples each) keep a fine-grained
    software pipeline across PE/ACT/DVE.
  - softmax denominators are computed TRANSPOSED (lhsT=p0, rhs=ones ->
    [128,1] per sub, batched in one [128,4] bank) so a single DVE
    reciprocal runs across 128 lanes; r broadcasts back to rows via an
    identity-rhs matmul with a stride-0 lhsT.
  - GEMM2 is one 512-col matmul per channel group j; its bias rides the
    ACT PSUM->SBUF copy; the residual is a pure bf16 SBUF tensor_tensor
    add at DVE 2x mode, done per (j, n-half) so each half's store (split
    across both HWDGE queues) starts early.
"""

import os
import sys

for _p in ("/opt/trn_rl_repo",):
    if _p not in sys.path:
        sys.path.insert(0, _p)

import numpy as np
from ml_dtypes import bfloat16

import concourse.mybir as mybir
from concourse import bacc, tile

EPS = 1e-5
N_TOTAL, C, D, HH, WW = 2048, 512, 64, 4, 4
NCORES = 8
NSH = N_TOTAL // NCORES  # 256 samples per core
BLK = int(os.environ.get("KBLK", "128"))  # samples per block
SUB = 32                 # samples per attention subchunk (4*SUB = 128 cols)
SHIFT = -34.0            # constant exp shift; cancels in the normalization
F32 = mybir.dt.float32
BF16 = mybir.dt.bfloat16

# bf16 const-blob column layout
_WQ, _WK, _W2A = 0, 256, 512
_MSK, _EYE, _W4T, _B2A, _CBW = 768, 896, 1024, 1536, 1600

_PROG_CACHE = {}


def build_program(nsh=NSH, blk=BLK, reps=1):
    key = (nsh, blk, reps)
    if key in _PROG_CACHE:
        return _PROG_CACHE[key]

    nc = bacc.Bacc("TRN2", target_bir_lowering=False, debug=False)
    AF = mybir.ActivationFunctionType

    x_in = nc.dram_tensor("x", (128, nsh * 64), BF16, kind="ExternalInput")
    cb = nc.dram_tensor("cb", (128, _CBW), BF16, kind="ExternalInput")
    cf = nc.dram_tensor("cf", (128, 8), F32, kind="ExternalInput")
    out = nc.dram_tensor("out", (128, nsh * 64), BF16, kind="ExternalOutput")

    nblk = nsh // blk
    nsub = blk // SUB
    NF = 4 * blk  # free width of a full block of (n, l) columns
    NH = blk // 2  # samples per store half

    with tile.TileContext(nc) as tc:
        with (
            tc.tile_pool(name="const", bufs=1) as cpool,
            tc.tile_pool(name="xp", bufs=(3 if blk >= 128 else 4)) as xpool,
            tc.tile_pool(name="work", bufs=4) as wpool,
            tc.tile_pool(name="att", bufs=6) as apool,
            tc.tile_pool(name="ps", bufs=5, space="PSUM") as pspool,
            tc.tile_pool(name="psd", bufs=1, space="PSUM") as pdpool,
            tc.tile_pool(name="psy", bufs=2, space="PSUM") as pypool,
        ):
            # constants first (small, gate all matmuls): scalar queue
            cb_sb = cpool.tile([128, _CBW], BF16)
            nc.scalar.dma_start(cb_sb[:], cb[:])
            cf_sb = cpool.tile([128, 8], F32)
            nc.scalar.dma_start(cf_sb[:], cf[:])

            # x block loads: block 0 split across both queues; later
            # blocks go on the scalar queue so they never get ahead of
            # block 0's halves in the HBM bandwidth race
            xv = x_in[:].rearrange("p (b n f) -> b p n f", n=blk, f=64)
            ov = out[:].rearrange("p (b n f) -> b p n f", n=blk, f=64)
            blist = [b for _ in range(reps) for b in range(nblk)]
            xts = {}
            for i, b in enumerate(blist[: max(2, min(3, len(blist)))]):
                x_t = xpool.tile([128, blk, 64], BF16, tag="x")
                if i == 0:
                    nc.sync.dma_start(x_t[:, 0:blk // 2], xv[b][:, 0:blk // 2])
                    nc.scalar.dma_start(x_t[:, blk // 2:], xv[b][:, blk // 2:])
                else:
                    nc.scalar.dma_start(x_t[:], xv[b])
                xts[i] = x_t

            ones_sb = cpool.tile([1, 128], BF16)
            nc.vector.memset(ones_sb[:], 1.0)
            ones_col = cpool.tile([128, 1], BF16)
            nc.vector.memset(ones_col[:], 1.0)
            shift_sb = cpool.tile([128, 1], F32)
            nc.vector.memset(shift_sb[:], SHIFT)

            # PE warm-up: the HAM clock gate keeps the PE at 1.2 GHz until
            # it has seen ~3.4us of sustained matmul activity.  While the
            # x/weight DMAs are in flight the PE is otherwise idle, so burn
            # that window on dummy matmuls over a memset tile; the real
            # GEMMs then start at 2.4 GHz.
            warm_in = cpool.tile([1, 512], BF16)
            nc.vector.memset(warm_in[:], 0.0)
            ps_w = pypool.tile([128, 512], F32, tag="psy")
            for k in range(26):
                nc.tensor.matmul(
                    ps_w[:, 0:(512 if k < 8 else 256)],
                    lhsT=ones_sb[:],
                    rhs=warm_in[:, 0:(512 if k < 8 else 256)],
                    start=True, stop=True,
                )

            wqv = cb_sb[:, _WQ:_WQ + 256].rearrange("p (j d) -> p j d", j=4)
            wkv = cb_sb[:, _WK:_WK + 256].rearrange("p (j d) -> p j d", j=4)
            w2v = cb_sb[:, _W2A:_W2A + 256].rearrange("p (j d) -> p j d", j=4)
            mskv = cb_sb[:, _MSK:_MSK + 128]
            eyev = cb_sb[:, _EYE:_EYE + 128]
            w4v = cb_sb[0:D, _W4T:_W4T + 512].rearrange(
                "p (j c) -> p j c", j=4)
            b2av = cb_sb[0:1, _B2A:_B2A + D]
            bq_ap = cf_sb[0:D, 0:1]
            bk_ap = cf_sb[0:D, 1:2]

            # ---- per-block stages, software-pipelined across blocks ----

            def st_front_a(i, b):
                """Load view, gather, GEMM1 q/k + relu."""
                if i in xts:
                    x_t = xts[i]
                else:
                    x_t = xpool.tile([128, blk, 64], BF16, tag="x")
                    nc.sync.dma_start(x_t[:], xv[b])
                # in-run order is (j, w, h)
                xtv = x_t[:].rearrange("p n (j w h) -> p n j w h", j=4, w=4)
                # gather the ::2,::2 columns -> [128, j, n, l], l=(h',w')
                xr = wpool.tile([128, 4, blk, 4], BF16, tag="xr")
                xrv = xr[:].rearrange("p j n (a c) -> p j n a c", a=2)
                for hh in range(2):
                    nsl = slice(hh * NH, (hh + 1) * NH)
                    nc.vector.tensor_copy(
                        xrv[:, :, nsl],
                        xtv[:, nsl, :, 0:4:2, 0:4:2].transpose(
                            [0, 2, 1, 4, 3]),
                    )
                xrf = xr[:].rearrange("p j n l -> p j (n l)")
                ps_q = pspool.tile([D, NF], F32, tag="ps")
                ps_k = pspool.tile([D, NF], F32, tag="ps")
                for j in range(4):
                    nc.tensor.matmul(
                        ps_q[:], lhsT=wqv[:, j], rhs=xrf[:, j],
                        start=(j == 0), stop=(j == 3),
                    )
                for j in range(4):
                    nc.tensor.matmul(
                        ps_k[:], lhsT=wkv[:, j], rhs=xrf[:, j],
                        start=(j == 0), stop=(j == 3),
                    )
                a_q = wpool.tile([D, NF], BF16, tag="aq")
                nc.scalar.activation(a_q[:], ps_q[:], AF.Relu, bias=bq_ap)
                a_k = wpool.tile([D, NF], BF16, tag="ak")
                nc.scalar.activation(a_k[:], ps_k[:], AF.Relu, bias=bk_ap)
                return dict(b=b, x_t=x_t, xtv=xtv, xrf=xrf, a_q=a_q, a_k=a_k)

            def st_front_b(st):
                """Phase 1: vt + gram matmuls for all subchunks."""
                xrf, a_q, a_k = st["xrf"], st["a_q"], st["a_k"]
                ph_vt, ph_g = [], []
                for s in range(nsub):
                    cl = slice(s * 128, s * 128 + 128)
                    ps_vt = pspool.tile([128, D], F32, tag="ps")
                    for j in range(4):
                        nc.tensor.matmul(
                            ps_vt[:], lhsT=xrf[:, j, cl], rhs=w2v[:, j],
                            start=(j == 0), stop=False,
                        )
                    nc.tensor.matmul(
                        ps_vt[:], lhsT=ones_sb[:], rhs=b2av,
                        start=False, stop=True,
                    )
                    ps_g = pspool.tile([128, 128], F32, tag="ps")
                    nc.tensor.matmul(
                        ps_g[:], lhsT=a_k[:, cl], rhs=a_q[:, cl],
                        start=True, stop=True,
                    )
                    ph_vt.append(ps_vt)
                    ph_g.append(ps_g)
                st["ph_vt"], st["ph_g"] = ph_vt, ph_g

            def st_attn(st):
                """Phase 2 + denominators + phase 3 -> normalized z."""
                ph_vt, ph_g = st["ph_vt"], st["ph_g"]
                ph_a2t, ph_p0 = [], []
                for s in range(nsub):
                    a2t = apool.tile([128, D], BF16, tag="a2t")
                    nc.scalar.activation(a2t[:], ph_vt[s][:], AF.Relu)
                    e_t = apool.tile([128, 128], BF16, tag="e")
                    nc.scalar.activation(e_t[:], ph_g[s][:], AF.Exp,
                                         bias=shift_sb[:])
                    p0 = apool.tile([128, 128], BF16, tag="p0")
                    nc.vector.tensor_mul(p0[:], e_t[:], mskv)
                    ph_a2t.append(a2t)
                    ph_p0.append(p0)
                ps_dt = pdpool.tile([128, nsub], F32, tag="psd")
                for s in range(nsub):
                    nc.tensor.matmul(
                        ps_dt[:, s:s + 1], lhsT=ph_p0[s][:], rhs=ones_col[:],
                        start=(s == 0), stop=(s == nsub - 1),
                    )
                rt_sb = apool.tile([128, nsub], BF16, tag="rt")
                with nc.allow_low_precision("bf16 softmax normalization"):
                    nc.vector.reciprocal(rt_sb[:], ps_dt[:])
                z_blk = apool.tile([D, NF], BF16, tag="z")
                for s in range(nsub):
                    a2t, p0 = ph_a2t[s], ph_p0[s]
                    ps_z = pspool.tile([D, 128], F32, tag="ps")
                    nc.tensor.matmul(
                        ps_z[:], lhsT=a2t[:], rhs=p0[:], start=True, stop=True,
                    )
                    ps_r = pspool.tile([D, 128], F32, tag="ps")
                    nc.tensor.matmul(
                        ps_r[:],
                        lhsT=rt_sb[:, s:s + 1].broadcast_to((128, D)),
                        rhs=eyev,
                        start=True, stop=True,
                    )
                    r64_sb = apool.tile([D, 128], BF16, tag="r64")
                    nc.scalar.activation(r64_sb[:], ps_r[:], AF.Copy)
                    nc.vector.tensor_mul(
                        z_blk[:, s * 128:(s + 1) * 128], ps_z[:], r64_sb[:])
                st["z_blk"] = z_blk

            def st_out(st):
                """GEMM2 + bias copy + residual + stores."""
                b, x_t, xtv, z_blk = st["b"], st["x_t"], st["xtv"], st["z_blk"]
                y4 = apool.tile([128, 4, NF], BF16, tag="y")
                for j in range(4):
                    ps_y = pypool.tile([128, NF], F32, tag="psy")
                    nc.tensor.matmul(
                        ps_y[:], lhsT=w4v[:, j], rhs=z_blk[:],
                        start=True, stop=True,
                    )
                    nc.scalar.activation(y4[:, j], ps_y[:], AF.Identity,
                                         bias=cf_sb[:, 4 + j:5 + j])
                y4v = y4[:].rearrange("p j (n l) -> p n j l", l=4)
                for hh in range(2):
                    nsl = slice(hh * NH, (hh + 1) * NH)
                    for j in range(4):
                        yv = (
                            y4v[:, nsl, j]
                            .unsqueeze(2)
                            .broadcast_to((128, NH, 4, 4))
                        )
                        nc.vector.tensor_add(xtv[:, nsl, j], yv,
                                             xtv[:, nsl, j])
                    eng = nc.sync if hh == 0 else nc.scalar
                    eng.dma_start(ov[b][:, nsl], x_t[:, nsl])

            # pipelined emission: block i+1's front overlaps block i's
            # attention/output stages
            sts = {}
            for i, b in enumerate(blist):
                if i == 0:
                    sts[0] = st_front_a(0, b)
                    st_front_b(sts[0])
                if i + 1 < len(blist):
                    sts[i + 1] = st_front_a(i + 1, blist[i + 1])
                st_attn(sts[i])
                if i + 1 < len(blist):
                    st_front_b(sts[i + 1])
                st_out(sts[i])
                del sts[i]

    nc.compile()
    _PROG_CACHE[key] = nc
    return nc


def _bf16(a):
    return np.ascontiguousarray(np.asarray(a, np.float32)).astype(bfloat16)


def prep_params(W123, b123, g123, be123, m123, v123, W4, b4, g4, be4, m4, v4):
    """Fold BN into the convs; pack into the two const blobs."""
    f64 = np.float64
    s123 = (g123 / np.sqrt(v123 + EPS)).astype(f64)            # (3, D)
    Wf = (W123 * s123[:, :, None]).astype(f64)                 # (3, D, C)
    bf = ((b123 - m123) * s123 + be123).astype(np.float32)     # (3, D)
    s4 = (g4 / np.sqrt(v4 + EPS)).astype(f64)                  # (C,)
    W4f = (W4 * s4[:, None]).astype(f64)                       # (C, D)
    b4f = ((b4 - m4) * s4 + be4).astype(np.float32)            # (C,)

    # perm[j*128 + p] = 4p + j : row j*128+p of a device weight tensor
    # holds original channel 4p+j (matching the x packing).
    p_idx, j_idx = np.meshgrid(np.arange(128), np.arange(4), indexing="ij")
    perm = (4 * p_idx + j_idx).T.reshape(-1)                   # (512,)

    def to_pjd(w):  # (D, C) weights -> [128, (j, D)] with c = 4p+j
        return w.T[perm].reshape(4, 128, D).transpose(1, 0, 2).reshape(128, -1)

    msk = np.kron(np.eye(SUB, dtype=np.float32), np.ones((4, 4), np.float32))
    cbv = np.zeros((128, _CBW), np.float32)
    cbv[:, _WQ:_WQ + 256] = to_pjd(Wf[0])
    cbv[:, _WK:_WK + 256] = to_pjd(Wf[1])
    cbv[:, _W2A:_W2A + 256] = to_pjd(Wf[2])
    cbv[:, _MSK:_MSK + 128] = msk
    cbv[:, _EYE:_EYE + 128] = np.eye(128, dtype=np.float32)
    cbv[0:D, _W4T:_W4T + 512] = W4f.T[:, perm].reshape(D, 512)
    cbv[0, _B2A:_B2A + D] = bf[2]
    cfv = np.zeros((128, 8), np.float32)
    cfv[0:D, 0] = bf[0]
    cfv[0:D, 1] = bf[1]
    cfv[:, 4:8] = b4f[perm].reshape(4, 128).T
    return dict(cb=_bf16(cbv), cf=cfv)


def pack_x(x):
    """(N, C, 4, 4) fp32 -> [NCORES][128, NSH*64] bf16.

    c = 4p+j on partitions; per-(p, n) run holds (j, w, h)."""
    xb = np.asarray(x, np.float32).reshape(NCORES, NSH, 128, 4, 4, 4)
    xb = xb.astype(bfloat16).transpose(0, 2, 1, 3, 5, 4)  # core p n j w h
    return np.ascontiguousarray(xb).reshape(NCORES, 128, NSH * 64)


def unpack_out(outs):
    """[NCORES][128, NSH*64] bf16 -> (N, C, 4, 4) fp32."""
    o = np.stack([np.asarray(c) for c in outs]).reshape(
        NCORES, 128, NSH, 4, 4, 4)               # core p n j w h
    o = o.transpose(0, 2, 1, 3, 5, 4)            # core n p j h w
    return np.ascontiguousarray(o).reshape(N_TOTAL, C, HH, WW).astype(
        np.float32)


def _run(inputs, trace=False, **spmd_kwargs):
    from concourse.bass_utils import run_bass_kernel_spmd

    xp = pack_x(inputs["x"])
    params = prep_params(**{k: np.asarray(v, np.float64)
                            for k, v in inputs.items() if k != "x"})
    nc = build_program()
    in_maps = [{"x": xp[i], **params} for i in range(NCORES)]
    res = run_bass_kernel_spmd(
        nc, in_maps, list(range(NCORES)), trace=trace, **spmd_kwargs
    )
    outs = unpack_out([res.results[i]["out"] for i in range(NCORES)])
    return outs, res


def kernel(**inputs):
    outs, _ = _run(inputs)
    return outs
